# revision 20
# baseline (speedup 1.0000x reference)
"""2-layer GCN (segment-sum message passing) on 8 trn2 NeuronCores.

Math (from the reference):
    row/col have self-loops appended with weight 5 (= trunc(log2(E/N))).
    deg[i] = (# real edges with row==i) + 5 ;  dis = 1/sqrt(deg)
    gcn(h, W): agg[d] = sum_{e: row=d} norm[e] * (h@W)[col[e]]
    out = (relu(gcn(x,W1)) -> gcn(.,W2)) @ W3.T + b3

Factorization: with zs = dis * (h@W),
    gcn(h,W)[d] = dis[d] * ( sum_{e: row=d} zs[col[e]]  +  5*zs[d] )
Per-edge work is a gather of zs rows plus a segment-sum on the tensor
engine, TRANSPOSED: per 128-edge chunk,
    aggT[f, d] += msg[e, f]^T(as lhsT) @ Sel[e, d]
with Sel the one-hot of each edge's dst-within-tile, built ON-CHIP by a
single DVE tensor_tensor(is_equal) per gather call (iota broadcast vs
dloc broadcast).  The transposed accumulator feeds W2/W3 matmuls directly
(lhsT = aggT) so no PE transposes are needed; dis scales fold into the
epilogue matmul outputs (zs2 = dis^2 * (relu(agg1) @ W2)).

Gathers run as SWDGE prepare_only + trigger_dma so the GpSimd engine only
pays descriptor-generation time (the baseline's dma_gather held the Q7
engine until the DMA drained, serializing the whole kernel).

Sharding: destination nodes split across 8 cores; zs tables are
all-gathered between layers in 4 quarter-slices (pipelined with the
gathers that consume them); gathers read the fp16 table via dma_gather
(int16 indices; buckets = table quarters to stay under 32768 rows).
"""

import math
import os
import sys

sys.path.insert(0, "/opt/trn_rl_repo")

import numpy as np

import concourse.bass as bass  # noqa: F401
import concourse.tile as tile
from concourse import bacc, mybir
from concourse.bass_utils import run_bass_kernel_spmd

NCORES = 8
SG_TILES = 3  # dst tiles per supergroup (2 PSUM sets of 3 -> cross-sg overlap)
NQ = 4  # table quarters (= gather buckets = sub-allgathers)
F16 = mybir.dt.float16
F8 = mybir.dt.float8e4
F32 = mybir.dt.float32


def _ceil(a, b):
    return -(-a // b)


def preprocess(x, edge_index, W1, W2, W3, b3):
    """Host-side layout/index prep. Returns (meta, in_maps)."""
    N, F_IN = x.shape
    HID = W1.shape[1]
    NCLS = W3.shape[0]
    E = edge_index.shape[1]

    npc_raw = _ceil(N, NCORES)  # nodes per core (unpadded)
    NPC = _ceil(npc_raw, 128) * 128  # padded per-core rows
    NTILES = NPC // 128
    NTAB = NCORES * NPC  # table rows
    NSG = _ceil(NTILES, SG_TILES)
    KIN = F_IN // 128  # contraction chunks for layer-1 matmul

    # table quarters on supergroup boundaries
    base, rem = divmod(NSG, NQ)
    qsg = [base + (1 if i < rem else 0) for i in range(NQ)]  # sgs per quarter
    qsg_start = np.concatenate(([0], np.cumsum(qsg)))  # sg index bounds
    qtile_start = np.minimum(qsg_start * SG_TILES, NTILES)
    qtiles = np.diff(qtile_start)  # tiles per quarter
    Q_local = qtiles * 128  # per-core rows per quarter
    assert Q_local.sum() == NPC
    QROWS = NCORES * Q_local  # table rows per quarter (bucket)
    assert QROWS.max() <= 32768  # int16 gather index limit
    toff = np.concatenate(([0], np.cumsum(QROWS)))  # table offset per quarter
    quarter_of_tile = np.searchsorted(qtile_start[1:], np.arange(NTILES), "right")

    row = np.asarray(edge_index[0], np.int64)
    col = np.asarray(edge_index[1], np.int64)

    fill = float(math.trunc(math.log2(E / N)))
    deg = np.bincount(row, minlength=N).astype(np.float64) + fill
    dis = (1.0 / np.sqrt(deg)).astype(np.float32)
    dis_pad = np.zeros(NTAB, np.float32)
    tab_row_of_node = (np.arange(N) // npc_raw) * NPC + (np.arange(N) % npc_raw)
    dis_pad[tab_row_of_node] = dis

    # per-edge placement (dst side)
    e_core = row // npc_raw
    e_local = row % npc_raw
    e_tile = e_local // 128
    e_dstloc = (e_local % 128).astype(np.float16)
    # src side -> quarter-major table position
    s_core = col // npc_raw
    s_local = col % npc_raw
    s_tile = s_local // 128
    s_q = quarter_of_tile[s_tile]
    e_bucket = s_q
    e_idx16 = (s_core * Q_local[s_q] + (s_local - 128 * qtile_start[s_q])).astype(
        np.int16
    )

    # stream order per core: supergroup -> bucket -> tile-within-sg -> edges
    e_sg = e_tile // SG_TILES
    e_tsg = e_tile % SG_TILES
    group = ((e_sg * NQ + e_bucket) * SG_TILES + e_tsg).astype(np.int64)
    NGRP = NSG * NQ * SG_TILES

    # counts[c, g] -> uniform padded counts across cores (SPMD: one program)
    flat = e_core * NGRP + group
    bc = np.bincount(flat, minlength=NCORES * NGRP)
    counts = bc.reshape(NCORES, NGRP)
    gcnt = counts.max(axis=0)
    gchunks = _ceil(gcnt, 128)  # chunks per group (uniform)
    gpad = gchunks * 128

    goff = np.zeros(NGRP + 1, np.int64)
    np.cumsum(gpad, out=goff[1:])
    TOT_IDX = int(goff[-1])
    TOT_CHUNKS = TOT_IDX // 128

    # assemble per-core streams
    order = np.argsort(flat, kind="stable")
    run_starts = np.concatenate(([0], np.cumsum(bc)[:-1]))
    within = np.empty(E, np.int64)
    within[order] = np.arange(E) - run_starts[flat[order]]
    pos = goff[group] + within
    idx_stream = np.zeros((NCORES, TOT_IDX), np.int16)
    dloc_stream = np.full((NCORES, TOT_IDX), -64.0, np.float16)
    idx_stream[e_core, pos] = e_idx16
    dloc_stream[e_core, pos] = e_dstloc

    # gather calls: contiguous (sg, bucket) runs of SG_TILES groups, split
    # so one call covers at most MAXCH chunks (walrus 8192-idx cap).
    MAXCH = 32
    calls = []  # (sg, bucket, idx_off, chunk_off, [(tile, nchunks), ...])
    for s in range(NSG):
        tiles = list(range(s * SG_TILES, min(NTILES, (s + 1) * SG_TILES)))
        for b in range(NQ):
            g0 = (s * NQ + b) * SG_TILES
            idx_off = int(goff[g0])
            assert idx_off % 128 == 0
            pieces = []
            cur = []
            cur_n = 0
            for i, t in enumerate(tiles):
                n = int(gchunks[g0 + i])
                while n > 0:
                    take = min(n, MAXCH - cur_n)
                    if take > 0:
                        cur.append((t, take))
                        cur_n += take
                        n -= take
                    if cur_n == MAXCH:
                        pieces.append(cur)
                        cur = []
                        cur_n = 0
            if cur:
                pieces.append(cur)
            off = idx_off
            for tl in pieces:
                calls.append((s, b, off, off // 128, tl))
                off += sum(nn for _, nn in tl) * 128
            assert off == int(goff[g0 + len(tiles)])

    # wrapped idx layout: per call, idx j -> [j%16, call_off/16 + j//16],
    # replicated to 128 partitions (8x). Call lengths are %128 so the wrap
    # works within the whole stream.
    idx_wrapped = np.zeros((NCORES, 128, TOT_IDX // 16), np.int16)
    for s, b, io, co, tl in calls:
        L = sum(n for _, n in tl) * 128
        if L == 0:
            continue
        blk = idx_stream[:, io : io + L].reshape(NCORES, L // 16, 16)
        blk = np.swapaxes(blk, 1, 2)  # [NCORES, 16, L/16]
        idx_wrapped[:, :, io // 16 : (io + L) // 16] = np.tile(blk, (1, 8, 1))

    dlocT = np.ascontiguousarray(
        np.swapaxes(dloc_stream.reshape(NCORES, TOT_CHUNKS, 128), 1, 2)
    )  # [NCORES, 128, TOT_CHUNKS] fp16

    # per-tile chunk totals (for start/stop flags)
    tile_chunks = np.zeros(NTILES, np.int64)
    for s, b, io, co, tl in calls:
        for t, n in tl:
            tile_chunks[t] += n

    # dense inputs
    xpad = np.zeros((NTAB, F_IN), np.float32)
    xpad[tab_row_of_node] = np.asarray(x, np.float32)
    xT = np.ascontiguousarray(
        np.swapaxes(xpad.reshape(NCORES, NPC, F_IN), 1, 2).astype(np.float16)
    )  # [NCORES, F_IN, NPC]

    disT = np.ascontiguousarray(
        np.swapaxes(dis_pad.reshape(NCORES, NTILES, 128), 1, 2)
    )  # [NCORES, 128, NTILES]
    dis2T = disT * disT

    iota_np = np.tile(np.arange(128, dtype=np.float16)[None, :], (128, 1))
    diag5_np = (fill * np.eye(128)).astype(mybir.dt.np(F8))
    W1_np = np.ascontiguousarray(
        np.asarray(W1, np.float32).reshape(KIN, 128, HID).astype(np.float16)
    )
    W2_np = np.asarray(W2, np.float32).astype(np.float16)
    W3T_np = np.ascontiguousarray(np.asarray(W3, np.float32).T.astype(np.float16))
    b3_rep_np = np.tile(np.asarray(b3, np.float32)[None, :], (128, 1))

    meta = dict(
        N=N,
        F_IN=F_IN,
        HID=HID,
        NCLS=NCLS,
        NPC=NPC,
        npc_raw=npc_raw,
        NTILES=NTILES,
        NTAB=NTAB,
        NSG=NSG,
        KIN=KIN,
        TOT_IDX=TOT_IDX,
        TOT_CHUNKS=TOT_CHUNKS,
        MAXCH=MAXCH,
        calls=calls,
        tile_chunks=tile_chunks,
        qsg_start=qsg_start,
        qtile_start=qtile_start,
        Q_local=Q_local,
        QROWS=QROWS,
        toff=toff,
    )
    in_maps = []
    for c in range(NCORES):
        in_maps.append(
            {
                "xT": xT[c],
                "idx": idx_wrapped[c],
                "dloc": dlocT[c],
                "iota": iota_np,
                "diag5": diag5_np,
                "dis": disT[c],
                "dis2": dis2T[c],
                "W1": W1_np,
                "W2": W2_np,
                "W3T": W3T_np,
                "b3rep": b3_rep_np,
            }
        )
    return meta, in_maps


def build_nc(meta):
    NPC = meta["NPC"]
    NTILES = meta["NTILES"]
    NTAB = meta["NTAB"]
    NSG = meta["NSG"]
    KIN = meta["KIN"]
    F_IN = meta["F_IN"]
    HID = meta["HID"]
    NCLS = meta["NCLS"]
    TOT_IDX = meta["TOT_IDX"]
    TOT_CHUNKS = meta["TOT_CHUNKS"]
    calls = meta["calls"]
    tile_chunks = meta["tile_chunks"]
    qsg_start = meta["qsg_start"]
    qtile_start = meta["qtile_start"]
    Q_local = meta["Q_local"]
    toff = meta["toff"]

    use_prep = os.environ.get("GCN_PREP", "0") == "1"

    nc = bacc.Bacc(
        "TRN2",
        target_bir_lowering=False,
        debug=False,
        num_devices=NCORES,
        dynamic_dma_scratch_size=65536,
        num_swdge_queues=4,
    )

    xT_d = nc.dram_tensor("xT", [F_IN, NPC], F16, kind="ExternalInput")
    idx_d = nc.dram_tensor(
        "idx", [128, TOT_IDX // 16], mybir.dt.int16, kind="ExternalInput"
    )
    dloc_d = nc.dram_tensor("dloc", [128, TOT_CHUNKS], F16, kind="ExternalInput")
    iota_d = nc.dram_tensor("iota", [128, 128], F16, kind="ExternalInput")
    diag5_d = nc.dram_tensor("diag5", [128, 128], F8, kind="ExternalInput")
    dis_d = nc.dram_tensor("dis", [128, NTILES], F32, kind="ExternalInput")
    dis2_d = nc.dram_tensor("dis2", [128, NTILES], F32, kind="ExternalInput")
    W1_d = nc.dram_tensor("W1", [KIN, 128, HID], F16, kind="ExternalInput")
    W2_d = nc.dram_tensor("W2", [HID, HID], F16, kind="ExternalInput")
    W3T_d = nc.dram_tensor("W3T", [HID, NCLS], F16, kind="ExternalInput")
    b3r_d = nc.dram_tensor("b3rep", [128, NCLS], F32, kind="ExternalInput")
    out_d = nc.dram_tensor("out", [NPC, NCLS], F32, kind="ExternalOutput")

    zs1_own = nc.dram_tensor("zs1_own", [NPC, HID], F16)
    zs2_own = nc.dram_tensor("zs2_own", [NPC, HID], F16)
    tab1 = nc.dram_tensor("tab1", [NTAB, HID], F16, addr_space="Shared")
    tab2 = nc.dram_tensor("tab2", [NTAB, HID], F16, addr_space="Shared")

    NQUEUES = int(os.environ.get("GCN_QUEUES", "4"))
    sem_per_prep = os.environ.get("GCN_SEM_PER_PREP", "0") == "1"
    qsems = [nc.alloc_semaphore(f"gsem{q}") for q in range(4)]
    psems = [nc.alloc_semaphore(f"prepsem{q}") for q in range(4)]
    prep_count = [0, 0, 0, 0]
    _semn = [0]

    def prep_sem(qn):
        if not sem_per_prep:
            return qsems[qn]
        _semn[0] += 1
        return nc.alloc_semaphore(f"psem{_semn[0]}")

    def sub_allgather(zs_own, tab, q):
        r0 = int(128 * qtile_start[q])
        rows = int(Q_local[q])
        if rows == 0:
            return
        nc.gpsimd.collective_compute(
            "AllGather",
            mybir.AluOpType.bypass,
            ins=[zs_own[r0 : r0 + rows, :]],
            outs=[tab[int(toff[q]) : int(toff[q + 1]), :]],
            replica_groups=[list(range(NCORES))],
        )

    with tile.TileContext(nc) as tc:
        with (
            tc.tile_pool(name="const", bufs=1) as constp,
            tc.tile_pool(name="zs", bufs=1) as zsp,
            tc.tile_pool(name="meta", bufs=8) as metap,
            tc.tile_pool(name="epi", bufs=3) as epip,
            tc.tile_pool(name="agg", bufs=SG_TILES, space="PSUM") as aggp,
            tc.tile_pool(name="mpsum", bufs=2, space="PSUM") as mpsump,
        ):
            # xin is scoped to phase A (closed before the big gat/sel pools
            # open) so its SBUF is reused.
            _xin_cm = tc.tile_pool(name="xin", bufs=2)
            xinp = _xin_cm.__enter__()
            iota_t = constp.tile([128, 128], F16)
            nc.sync.dma_start(iota_t[:], iota_d[:])
            diag5_t = constp.tile([128, 128], F8)
            nc.sync.dma_start(diag5_t[:], diag5_d[:])
            dis_t = constp.tile([128, NTILES], F32)
            nc.sync.dma_start(dis_t[:], dis_d[:])
            dis2_t = constp.tile([128, NTILES], F32)
            nc.sync.dma_start(dis2_t[:], dis2_d[:])
            dloc_t = constp.tile([128, TOT_CHUNKS], F16)
            nc.sync.dma_start(dloc_t[:], dloc_d[:])
            W1_t = constp.tile([128, KIN, HID], F16)
            nc.sync.dma_start(W1_t[:], W1_d.rearrange("k p h -> p k h"))
            W2_t = constp.tile([HID, HID], F16)
            nc.sync.dma_start(W2_t[:], W2_d[:])
            W3T_t = constp.tile([HID, NCLS], F16)
            nc.sync.dma_start(W3T_t[:], W3T_d[:])
            b3r_t = constp.tile([128, NCLS], F32)
            nc.sync.dma_start(b3r_t[:], b3r_d[:])

            pending_ag = []
            zs1_all = zsp.tile([128, NTILES, HID], F16, tag="zs1")
            zs2_all = zsp.tile([128, NTILES, HID], F16, tag="zs2")

            xT_v = xT_d.rearrange("(k p) n -> k p n", p=128)
            zs1_v = zs1_own.rearrange("(g p) h -> g p h", p=128)
            zs2_v = zs2_own.rearrange("(g p) h -> g p h", p=128)
            out_v = out_d.rearrange("(g p) c -> g p c", p=128)

            # ---------------- phase A: zs1 = dis * (x @ W1) ----------------
            for s in range(NSG):
                t0 = s * SG_TILES
                nt = min(NTILES, t0 + SG_TILES) - t0
                xs = xinp.tile([128, KIN, SG_TILES * 128], F16, tag="xs")
                nc.sync.dma_start(
                    xs[:, :, : nt * 128],
                    xT_v[:, :, t0 * 128 : (t0 + nt) * 128].rearrange(
                        "k p n -> p k n"
                    ),
                )
                for i in range(nt):
                    t = t0 + i
                    z_ps = mpsump.tile([128, HID], F32, tag="mm")
                    for k in range(KIN):
                        nc.tensor.matmul(
                            z_ps[:],
                            xs[:, k, i * 128 : (i + 1) * 128],
                            W1_t[:, k, :],
                            start=(k == 0),
                            stop=(k == KIN - 1),
                        )
                    nc.scalar.activation(
                        zs1_all[:, t, :],
                        z_ps[:],
                        mybir.ActivationFunctionType.Copy,
                        bias=0.0,
                        scale=dis_t[:, t : t + 1],
                    )
                nc.sync.dma_start(
                    zs1_v[t0 : t0 + nt].rearrange("g p h -> p g h"),
                    zs1_all[:, t0 : t0 + nt, :],
                )
                # quarter complete -> pipelined sub-allgather (the last
                # quarter's AG is deferred into layer 1's first supergroup so
                # its sem wait doesn't stall the first gather calls)
                for q in range(NQ):
                    if s == qsg_start[q + 1] - 1:
                        if q == NQ - 1 and os.environ.get("GCN_PAG", "1") == "1":
                            pending_ag.append(
                                lambda q=q: sub_allgather(zs1_own, tab1, q)
                            )
                        else:
                            sub_allgather(zs1_own, tab1, q)

            _xin_cm.__exit__(None, None, None)
            _gat_cm = tc.tile_pool(name="gat", bufs=8)
            gatp = _gat_cm.__enter__()
            _sel_cm = tc.tile_pool(name="sel", bufs=6)
            selp = _sel_cm.__enter__()

            # ---------------- agg layer (shared for both layers) -----------
            psums = {}
            delayed_ag = {}

            def agg_layer(tab_dram, zs_src_all, layer):
                parts = set(
                    os.environ.get("GCN_AGG_PARTS", "gather,sel,mm,epi").split(",")
                )
                tile_seen = np.zeros(NTILES, np.int64)
                qn = 0
                for s in range(NSG):
                    t0 = s * SG_TILES
                    nt = min(NTILES, t0 + SG_TILES) - t0
                    # self-loop first (opens accumulation): aggT += zsT @ 5I
                    for i in range(nt):
                        t = t0 + i
                        ps = aggp.tile([128, 128], F32, tag="agg")
                        psums[t] = ps
                        nc.tensor.matmul(
                            ps[:],
                            zs_src_all[:, t, :],
                            diag5_t[:],
                            start=True,
                            stop=("mm" not in parts),
                        )
                    sgcalls = [c for c in calls if c[0] == s]
                    for ci, (_, b, io, co, tl) in enumerate(sgcalls):
                        if s == 0 and ci == len(sgcalls) - 1 and pending_ag:
                            pending_ag.pop()()
                        nch = sum(n for _, n in tl)
                        if nch == 0:
                            continue
                        L = nch * 128
                        idx_t = metap.tile(
                            [128, L // 16], mybir.dt.int16, tag="idx"
                        )
                        nc.sync.dma_start(
                            idx_t[:], idx_d[:, io // 16 : (io + L) // 16]
                        )
                        # Sel one-hot for the whole call in one DVE op:
                        # sel[p, j, d] = (dloc[p, co+j] == iota[d])
                        sel_t = selp.tile([128, 32, 128], F8, tag="sel")
                        if "sel" in parts:
                            in0 = bass.AP(
                                iota_t[:].tensor,
                                iota_t[:].offset,
                                [iota_t[:].ap[0], [0, nch], iota_t[:].ap[1]],
                            )
                            dl = dloc_t[:, co : co + nch]
                            in1 = bass.AP(
                                dl.tensor, dl.offset, [dl.ap[0], dl.ap[1], [0, 128]]
                            )
                            nc.vector.tensor_tensor(
                                sel_t[:, :nch, :],
                                in0,
                                in1,
                                mybir.AluOpType.is_equal,
                            )
                        else:
                            nc.vector.memset(sel_t[:, :nch, :], 0.0)
                        msg_t = gatp.tile([128, nch, HID], F16, tag="msg")
                        if "gather" in parts:
                            r0 = int(toff[b])
                            r1 = int(toff[b + 1])
                            if use_prep:
                                nc.gpsimd.dma_gather(
                                    msg_t[:],
                                    tab_dram[r0:r1, :],
                                    idx_t[:],
                                    L,
                                    L,
                                    HID,
                                    single_packet=False,
                                    queue_num=qn,
                                    prepare_only=True,
                                    sem=prep_sem(qn),
                                ).then_inc(psems[qn], 1)
                                prep_count[qn] += 1
                                nc.gpsimd.wait_ge(psems[qn], prep_count[qn])
                                nc.gpsimd.trigger_dma(count=1, queue_num=qn)
                            else:
                                nc.gpsimd.dma_gather(
                                    msg_t[:],
                                    tab_dram[r0:r1, :],
                                    idx_t[:],
                                    L,
                                    L,
                                    HID,
                                    single_packet=os.environ.get(
                                        "GCN_SP", "0"
                                    )
                                    == "1",
                                    queue_num=qn,
                                )
                            qn = (qn + 1) % NQUEUES
                        else:
                            nc.vector.memset(msg_t[:], 0.0)
                        j = 0
                        for t, n in tl:
                            for _ in range(n):
                                tile_seen[t] += 1
                                if "mm" in parts:
                                    nc.tensor.matmul(
                                        psums[t][:],
                                        msg_t[:, j, :],
                                        sel_t[:, j, :],
                                        start=False,
                                        stop=(tile_seen[t] == tile_chunks[t]),
                                    )
                                j += 1
                    # epilogue for the PREVIOUS supergroup (software
                    # pipeline: keeps the DVE/ACT epilogue ops out of the
                    # critical DVE-sel / PE-matmul overlap window)
                    if s > 0 and "epi" in parts:
                        sg_epilogue(s - 1, layer)
                    for fn in delayed_ag.pop(s, []):
                        fn()
                if "epi" in parts:
                    sg_epilogue(NSG - 1, layer)
                for k in sorted(delayed_ag):
                    for fn in delayed_ag.pop(k):
                        fn()

            def sg_epilogue(s, layer):
                t0 = s * SG_TILES
                nt = min(NTILES, t0 + SG_TILES) - t0
                o_sg = None
                if layer == 2:
                    o_sg = epip.tile([128, SG_TILES, NCLS], F32, tag="o")
                for i in range(nt):
                    epilogue(t0 + i, i, psums.pop(t0 + i), layer, o_sg)
                if layer == 1:
                    nc.sync.dma_start(
                        zs2_v[t0 : t0 + nt].rearrange("g p h -> p g h"),
                        zs2_all[:, t0 : t0 + nt, :],
                    )
                    for q in range(NQ):
                        if s == qsg_start[q + 1] - 1:
                            if q == NQ - 1 and os.environ.get("GCN_PAG", "1") == "1":
                                pending_ag.append(
                                    lambda q=q: sub_allgather(zs2_own, tab2, q)
                                )
                            else:
                                # issue 2 sgs later so the collective's sem
                                # wait doesn't park the Pool engine
                                delayed_ag.setdefault(s + 2, []).append(
                                    lambda q=q: sub_allgather(zs2_own, tab2, q)
                                )
                else:
                    nc.sync.dma_start(
                        out_v[t0 : t0 + nt].rearrange("g p c -> p g c"),
                        o_sg[:, :nt, :],
                    )

            def epilogue(t, i, ps, layer, o_sg):
                if layer == 1:
                    # h1T' = relu(aggT); zs2 = dis^2 * (h1' @ W2)
                    h_sb = epip.tile([128, 128], F16, tag="h")
                    nc.vector.tensor_scalar_max(h_sb[:], ps[:], 0.0)
                    z_ps = mpsump.tile([128, HID], F32, tag="mm")
                    nc.tensor.matmul(z_ps[:], h_sb[:], W2_t[:])
                    nc.scalar.activation(
                        zs2_all[:, t, :],
                        z_ps[:],
                        mybir.ActivationFunctionType.Copy,
                        bias=0.0,
                        scale=dis2_t[:, t : t + 1],
                    )
                else:
                    # o = dis * (agg2T^T @ W3T) + b3
                    h_sb = epip.tile([128, 128], F16, tag="h")
                    nc.vector.tensor_copy(h_sb[:], ps[:])
                    o_ps = mpsump.tile([128, NCLS], F32, tag="mm")
                    nc.tensor.matmul(o_ps[:], h_sb[:], W3T_t[:])
                    nc.vector.scalar_tensor_tensor(
                        o_sg[:, i, :],
                        o_ps[:],
                        dis_t[:, t : t + 1],
                        b3r_t[:],
                        mybir.AluOpType.mult,
                        mybir.AluOpType.add,
                    )

            dbg = int(os.environ.get("GCN_DEBUG_LEVEL", "3"))
            zs2_q_dma = None  # noqa: F841

            if dbg >= 2:
                agg_layer(tab1, zs1_all, layer=1)

            if dbg >= 3:
                agg_layer(tab2, zs2_all, layer=2)
            else:
                zt = epip.tile([128, NCLS], F32, tag="zt")
                nc.vector.memset(zt[:], 0.0)
                for t in range(NTILES):
                    nc.sync.dma_start(out_d[t * 128 : (t + 1) * 128, :], zt[:])

            _sel_cm.__exit__(None, None, None)
            _gat_cm.__exit__(None, None, None)

    nc.compile()
    return nc


_PROFILE_HOOK_DONE = False


def _install_profile_hook():
    """The container's antenv lacks axon_hooks; inject it so trace=True works."""
    global _PROFILE_HOOK_DONE
    if _PROFILE_HOOK_DONE:
        return
    _PROFILE_HOOK_DONE = True
    import types

    try:
        from antenv.axon_hooks import get_axon_ntff_profile_hook  # noqa: F401

        return  # real module exists
    except ImportError:
        pass
    try:
        from trn_agent_boot.trn_boot import _ntff_profile_via_ctypes

        hook = _ntff_profile_via_ctypes("/opt/axon/libaxon_pjrt.so")
    except Exception:
        hook = None
    mod = types.ModuleType("antenv.axon_hooks")
    mod._hook = hook
    mod.set_axon_ntff_profile_hook = lambda h: setattr(mod, "_hook", h)
    mod.get_axon_ntff_profile_hook = lambda: mod._hook
    import antenv

    sys.modules["antenv.axon_hooks"] = mod
    antenv.axon_hooks = mod


def kernel(x, edge_index, W1, W2, W3, b3, trace=False):
    x = np.asarray(x)
    edge_index = np.asarray(edge_index)
    if trace:
        _install_profile_hook()
    meta, in_maps = preprocess(x, edge_index, W1, W2, W3, b3)
    nc = build_nc(meta)
    res = run_bass_kernel_spmd(nc, in_maps, list(range(NCORES)), trace=trace)
    outs = []
    for c in range(NCORES):
        o = res.results[c]["out"]  # [NPC, NCLS]
        outs.append(o[: meta["npc_raw"]])
    full = np.concatenate(outs, axis=0)[: meta["N"]]
    kernel.last_result = res
    return np.ascontiguousarray(full.astype(np.float32))


if __name__ == "__main__":
    # tiny self-test
    rng = np.random.default_rng(1)
    N, E, F, H, C = 2048, 16384, 512, 128, 16
    x = rng.standard_normal((N, F)).astype(np.float32)
    ei = rng.integers(0, N, (2, E)).astype(np.int32)
    W1 = (rng.standard_normal((F, H)) / np.sqrt(F)).astype(np.float32)
    W2 = (rng.standard_normal((H, H)) / np.sqrt(H)).astype(np.float32)
    W3 = (rng.standard_normal((C, H)) / np.sqrt(H)).astype(np.float32)
    b3 = np.zeros(C, np.float32)

    fill = float(np.trunc(np.log2(E / N)))
    deg = np.bincount(ei[0], minlength=N) + fill
    dis = 1.0 / np.sqrt(deg)

    def gcn(h, W):
        z = h @ W
        zs = dis[:, None] * z
        agg = np.zeros_like(zs)
        np.add.at(agg, ei[0], zs[ei[1]])
        return dis[:, None] * (agg + fill * zs)

    h = np.maximum(gcn(x, W1), 0.0)
    h = gcn(h, W2)
    expected = h @ W3.T + b3

    got = kernel(x, ei, W1, W2, W3, b3)
    err = np.abs(got - expected).max() / np.abs(expected).max()
    print(f"rel err: {err:.3e}")
    print("PASS" if err < 2e-2 else "FAIL")


# revision 22
# speedup vs baseline: 1.0285x; 1.0285x over previous
"""2-layer GCN (segment-sum message passing) on 8 trn2 NeuronCores.

Math (from the reference):
    row/col have self-loops appended with weight 5 (= trunc(log2(E/N))).
    deg[i] = (# real edges with row==i) + 5 ;  dis = 1/sqrt(deg)
    gcn(h, W): agg[d] = sum_{e: row=d} norm[e] * (h@W)[col[e]]
    out = (relu(gcn(x,W1)) -> gcn(.,W2)) @ W3.T + b3

Factorization: with zs = dis * (h@W),
    gcn(h,W)[d] = dis[d] * ( sum_{e: row=d} zs[col[e]]  +  5*zs[d] )
Per-edge work is a gather of zs rows plus a segment-sum on the tensor
engine, TRANSPOSED: per 128-edge chunk,
    aggT[f, d] += msg[e, f]^T(as lhsT) @ Sel[e, d]
with Sel the one-hot of each edge's dst-within-tile, built ON-CHIP by a
single DVE tensor_tensor(is_equal) per gather call (iota broadcast vs
dloc broadcast).  The transposed accumulator feeds W2/W3 matmuls directly
(lhsT = aggT) so no PE transposes are needed; dis scales fold into the
epilogue matmul outputs (zs2 = dis^2 * (relu(agg1) @ W2)).

Gathers run as SWDGE prepare_only + trigger_dma so the GpSimd engine only
pays descriptor-generation time (the baseline's dma_gather held the Q7
engine until the DMA drained, serializing the whole kernel).

Sharding: destination nodes split across 8 cores; zs tables are
all-gathered between layers in 4 quarter-slices (pipelined with the
gathers that consume them); gathers read the fp16 table via dma_gather
(int16 indices; buckets = table quarters to stay under 32768 rows).
"""

import math
import os
import sys

sys.path.insert(0, "/opt/trn_rl_repo")

import numpy as np

import concourse.bass as bass  # noqa: F401
import concourse.tile as tile
from concourse import bacc, mybir
from concourse.bass_utils import run_bass_kernel_spmd

NCORES = 8
SG_TILES = 3  # dst tiles per supergroup (2 PSUM sets of 3 -> cross-sg overlap)
NQ = 4  # table quarters (= gather buckets = sub-allgathers)
F16 = mybir.dt.float16
F8 = mybir.dt.float8e4
F32 = mybir.dt.float32


def _ceil(a, b):
    return -(-a // b)


def preprocess(x, edge_index, W1, W2, W3, b3):
    """Host-side layout/index prep. Returns (meta, in_maps)."""
    N, F_IN = x.shape
    HID = W1.shape[1]
    NCLS = W3.shape[0]
    E = edge_index.shape[1]

    npc_raw = _ceil(N, NCORES)  # nodes per core (unpadded)
    NPC = _ceil(npc_raw, 128) * 128  # padded per-core rows
    NTILES = NPC // 128
    NTAB = NCORES * NPC  # table rows
    NSG = _ceil(NTILES, SG_TILES)
    KIN = F_IN // 128  # contraction chunks for layer-1 matmul

    # table quarters on supergroup boundaries
    base, rem = divmod(NSG, NQ)
    qsg = [base + (1 if i < rem else 0) for i in range(NQ)]  # sgs per quarter
    qsg_start = np.concatenate(([0], np.cumsum(qsg)))  # sg index bounds
    qtile_start = np.minimum(qsg_start * SG_TILES, NTILES)
    qtiles = np.diff(qtile_start)  # tiles per quarter
    Q_local = qtiles * 128  # per-core rows per quarter
    assert Q_local.sum() == NPC
    QROWS = NCORES * Q_local  # table rows per quarter (bucket)
    assert QROWS.max() <= 32768  # int16 gather index limit
    toff = np.concatenate(([0], np.cumsum(QROWS)))  # table offset per quarter
    quarter_of_tile = np.searchsorted(qtile_start[1:], np.arange(NTILES), "right")

    row = np.asarray(edge_index[0], np.int64)
    col = np.asarray(edge_index[1], np.int64)

    fill = float(math.trunc(math.log2(E / N)))
    deg = np.bincount(row, minlength=N).astype(np.float64) + fill
    dis = (1.0 / np.sqrt(deg)).astype(np.float32)
    dis_pad = np.zeros(NTAB, np.float32)
    tab_row_of_node = (np.arange(N) // npc_raw) * NPC + (np.arange(N) % npc_raw)
    dis_pad[tab_row_of_node] = dis

    # per-edge placement (dst side)
    e_core = row // npc_raw
    e_local = row % npc_raw
    e_tile = e_local // 128
    e_dstloc = (e_local % 128).astype(np.float16)
    # src side -> quarter-major table position
    s_core = col // npc_raw
    s_local = col % npc_raw
    s_tile = s_local // 128
    s_q = quarter_of_tile[s_tile]
    e_bucket = s_q
    e_idx16 = (s_core * Q_local[s_q] + (s_local - 128 * qtile_start[s_q])).astype(
        np.int16
    )

    # stream order per core: supergroup -> bucket -> tile-within-sg -> edges
    e_sg = e_tile // SG_TILES
    e_tsg = e_tile % SG_TILES
    group = ((e_sg * NQ + e_bucket) * SG_TILES + e_tsg).astype(np.int64)
    NGRP = NSG * NQ * SG_TILES

    # counts[c, g] -> uniform padded counts across cores (SPMD: one program)
    flat = e_core * NGRP + group
    bc = np.bincount(flat, minlength=NCORES * NGRP)
    counts = bc.reshape(NCORES, NGRP)
    gcnt = counts.max(axis=0)
    gchunks = _ceil(gcnt, 128)  # chunks per group (uniform)
    gpad = gchunks * 128

    goff = np.zeros(NGRP + 1, np.int64)
    np.cumsum(gpad, out=goff[1:])
    TOT_IDX = int(goff[-1])
    TOT_CHUNKS = TOT_IDX // 128

    # assemble per-core streams
    order = np.argsort(flat, kind="stable")
    run_starts = np.concatenate(([0], np.cumsum(bc)[:-1]))
    within = np.empty(E, np.int64)
    within[order] = np.arange(E) - run_starts[flat[order]]
    pos = goff[group] + within
    idx_stream = np.zeros((NCORES, TOT_IDX), np.int16)
    dloc_stream = np.full((NCORES, TOT_IDX), -64.0, np.float16)
    idx_stream[e_core, pos] = e_idx16
    dloc_stream[e_core, pos] = e_dstloc

    # gather calls: contiguous (sg, bucket) runs of SG_TILES groups, split
    # so one call covers at most MAXCH chunks (walrus 8192-idx cap).
    MAXCH = 32
    calls = []  # (sg, bucket, idx_off, chunk_off, [(tile, nchunks), ...])
    for s in range(NSG):
        tiles = list(range(s * SG_TILES, min(NTILES, (s + 1) * SG_TILES)))
        for b in range(NQ):
            g0 = (s * NQ + b) * SG_TILES
            idx_off = int(goff[g0])
            assert idx_off % 128 == 0
            pieces = []
            cur = []
            cur_n = 0
            for i, t in enumerate(tiles):
                n = int(gchunks[g0 + i])
                while n > 0:
                    take = min(n, MAXCH - cur_n)
                    if take > 0:
                        cur.append((t, take))
                        cur_n += take
                        n -= take
                    if cur_n == MAXCH:
                        pieces.append(cur)
                        cur = []
                        cur_n = 0
            if cur:
                pieces.append(cur)
            off = idx_off
            for tl in pieces:
                calls.append((s, b, off, off // 128, tl))
                off += sum(nn for _, nn in tl) * 128
            assert off == int(goff[g0 + len(tiles)])

    # wrapped idx layout: per call, idx j -> [j%16, call_off/16 + j//16],
    # replicated to 128 partitions (8x). Call lengths are %128 so the wrap
    # works within the whole stream.
    idx_wrapped = np.zeros((NCORES, 128, TOT_IDX // 16), np.int16)
    for s, b, io, co, tl in calls:
        L = sum(n for _, n in tl) * 128
        if L == 0:
            continue
        blk = idx_stream[:, io : io + L].reshape(NCORES, L // 16, 16)
        blk = np.swapaxes(blk, 1, 2)  # [NCORES, 16, L/16]
        idx_wrapped[:, :, io // 16 : (io + L) // 16] = np.tile(blk, (1, 8, 1))

    dlocT = np.ascontiguousarray(
        np.swapaxes(dloc_stream.reshape(NCORES, TOT_CHUNKS, 128), 1, 2)
    )  # [NCORES, 128, TOT_CHUNKS] fp16

    # per-tile chunk totals (for start/stop flags)
    tile_chunks = np.zeros(NTILES, np.int64)
    for s, b, io, co, tl in calls:
        for t, n in tl:
            tile_chunks[t] += n

    # dense inputs
    xpad = np.zeros((NTAB, F_IN), np.float32)
    xpad[tab_row_of_node] = np.asarray(x, np.float32)
    xT = np.ascontiguousarray(
        np.swapaxes(xpad.reshape(NCORES, NPC, F_IN), 1, 2).astype(np.float16)
    )  # [NCORES, F_IN, NPC]

    disT = np.ascontiguousarray(
        np.swapaxes(dis_pad.reshape(NCORES, NTILES, 128), 1, 2)
    )  # [NCORES, 128, NTILES]
    dis2T = disT * disT

    iota_np = np.tile(np.arange(128, dtype=np.float16)[None, :], (128, 1))
    diag5_np = (fill * np.eye(128)).astype(mybir.dt.np(F8))
    W1_np = np.ascontiguousarray(
        np.asarray(W1, np.float32).reshape(KIN, 128, HID).astype(np.float16)
    )
    W2_np = np.asarray(W2, np.float32).astype(np.float16)
    W3T_np = np.ascontiguousarray(np.asarray(W3, np.float32).T.astype(np.float16))
    b3_rep_np = np.tile(np.asarray(b3, np.float32)[None, :], (128, 1))

    meta = dict(
        N=N,
        F_IN=F_IN,
        HID=HID,
        NCLS=NCLS,
        NPC=NPC,
        npc_raw=npc_raw,
        NTILES=NTILES,
        NTAB=NTAB,
        NSG=NSG,
        KIN=KIN,
        TOT_IDX=TOT_IDX,
        TOT_CHUNKS=TOT_CHUNKS,
        MAXCH=MAXCH,
        calls=calls,
        tile_chunks=tile_chunks,
        qsg_start=qsg_start,
        qtile_start=qtile_start,
        Q_local=Q_local,
        QROWS=QROWS,
        toff=toff,
    )
    in_maps = []
    for c in range(NCORES):
        in_maps.append(
            {
                "xT": xT[c],
                "idx": idx_wrapped[c],
                "dloc": dlocT[c],
                "iota": iota_np,
                "diag5": diag5_np,
                "dis": disT[c],
                "dis2": dis2T[c],
                "W1": W1_np,
                "W2": W2_np,
                "W3T": W3T_np,
                "b3rep": b3_rep_np,
            }
        )
    return meta, in_maps


def build_nc(meta):
    NPC = meta["NPC"]
    NTILES = meta["NTILES"]
    NTAB = meta["NTAB"]
    NSG = meta["NSG"]
    KIN = meta["KIN"]
    F_IN = meta["F_IN"]
    HID = meta["HID"]
    NCLS = meta["NCLS"]
    TOT_IDX = meta["TOT_IDX"]
    TOT_CHUNKS = meta["TOT_CHUNKS"]
    calls = meta["calls"]
    tile_chunks = meta["tile_chunks"]
    qsg_start = meta["qsg_start"]
    qtile_start = meta["qtile_start"]
    Q_local = meta["Q_local"]
    toff = meta["toff"]

    use_prep = os.environ.get("GCN_PREP", "0") == "1"

    nc = bacc.Bacc(
        "TRN2",
        target_bir_lowering=False,
        debug=False,
        num_devices=NCORES,
        dynamic_dma_scratch_size=98304,
        num_swdge_queues=4,
    )

    xT_d = nc.dram_tensor("xT", [F_IN, NPC], F16, kind="ExternalInput")
    idx_d = nc.dram_tensor(
        "idx", [128, TOT_IDX // 16], mybir.dt.int16, kind="ExternalInput"
    )
    dloc_d = nc.dram_tensor("dloc", [128, TOT_CHUNKS], F16, kind="ExternalInput")
    iota_d = nc.dram_tensor("iota", [128, 128], F16, kind="ExternalInput")
    diag5_d = nc.dram_tensor("diag5", [128, 128], F8, kind="ExternalInput")
    dis_d = nc.dram_tensor("dis", [128, NTILES], F32, kind="ExternalInput")
    dis2_d = nc.dram_tensor("dis2", [128, NTILES], F32, kind="ExternalInput")
    W1_d = nc.dram_tensor("W1", [KIN, 128, HID], F16, kind="ExternalInput")
    W2_d = nc.dram_tensor("W2", [HID, HID], F16, kind="ExternalInput")
    W3T_d = nc.dram_tensor("W3T", [HID, NCLS], F16, kind="ExternalInput")
    b3r_d = nc.dram_tensor("b3rep", [128, NCLS], F32, kind="ExternalInput")
    out_d = nc.dram_tensor("out", [NPC, NCLS], F32, kind="ExternalOutput")

    zs1_own = nc.dram_tensor("zs1_own", [NPC, HID], F16)
    zs2_own = nc.dram_tensor("zs2_own", [NPC, HID], F16)
    tab1 = nc.dram_tensor("tab1", [NTAB, HID], F16, addr_space="Shared")
    tab2 = nc.dram_tensor("tab2", [NTAB, HID], F16, addr_space="Shared")

    NQUEUES = int(os.environ.get("GCN_QUEUES", "4"))
    sem_per_prep = os.environ.get("GCN_SEM_PER_PREP", "0") == "1"
    qsems = [nc.alloc_semaphore(f"gsem{q}") for q in range(4)]
    psems = [nc.alloc_semaphore(f"prepsem{q}") for q in range(4)]
    prep_count = [0, 0, 0, 0]
    _semn = [0]

    def prep_sem(qn):
        if not sem_per_prep:
            return qsems[qn]
        _semn[0] += 1
        return nc.alloc_semaphore(f"psem{_semn[0]}")

    def sub_allgather(zs_own, tab, q):
        r0 = int(128 * qtile_start[q])
        rows = int(Q_local[q])
        if rows == 0:
            return
        nc.gpsimd.collective_compute(
            "AllGather",
            mybir.AluOpType.bypass,
            ins=[zs_own[r0 : r0 + rows, :]],
            outs=[tab[int(toff[q]) : int(toff[q + 1]), :]],
            replica_groups=[list(range(NCORES))],
        )

    with tile.TileContext(nc) as tc:
        with (
            tc.tile_pool(name="const", bufs=1) as constp,
            tc.tile_pool(name="zs", bufs=1) as zsp,
            tc.tile_pool(name="meta", bufs=6) as metap,
            tc.tile_pool(name="epi", bufs=3) as epip,
            tc.tile_pool(name="agg", bufs=SG_TILES, space="PSUM") as aggp,
            tc.tile_pool(name="mpsum", bufs=2, space="PSUM") as mpsump,
        ):
            # xin is scoped to phase A (closed before the big gat/sel pools
            # open) so its SBUF is reused.
            _xin_cm = tc.tile_pool(name="xin", bufs=2)
            xinp = _xin_cm.__enter__()
            iota_t = constp.tile([128, 128], F16)
            nc.sync.dma_start(iota_t[:], iota_d[:])
            diag5_t = constp.tile([128, 128], F8)
            nc.sync.dma_start(diag5_t[:], diag5_d[:])
            dis_t = constp.tile([128, NTILES], F32)
            nc.sync.dma_start(dis_t[:], dis_d[:])
            dis2_t = constp.tile([128, NTILES], F32)
            nc.sync.dma_start(dis2_t[:], dis2_d[:])
            dloc_t = constp.tile([128, TOT_CHUNKS], F16)
            nc.sync.dma_start(dloc_t[:], dloc_d[:])
            W1_t = constp.tile([128, KIN, HID], F16)
            nc.sync.dma_start(W1_t[:], W1_d.rearrange("k p h -> p k h"))
            W2_t = constp.tile([HID, HID], F16)
            nc.sync.dma_start(W2_t[:], W2_d[:])
            W3T_t = constp.tile([HID, NCLS], F16)
            nc.sync.dma_start(W3T_t[:], W3T_d[:])
            b3r_t = constp.tile([128, NCLS], F32)
            nc.sync.dma_start(b3r_t[:], b3r_d[:])

            pending_ag = []
            zs1_all = zsp.tile([128, NTILES, HID], F16, tag="zs1")
            zs2_all = zsp.tile([128, NTILES, HID], F16, tag="zs2")

            xT_v = xT_d.rearrange("(k p) n -> k p n", p=128)
            zs1_v = zs1_own.rearrange("(g p) h -> g p h", p=128)
            zs2_v = zs2_own.rearrange("(g p) h -> g p h", p=128)
            out_v = out_d.rearrange("(g p) c -> g p c", p=128)

            # ---------------- phase A: zs1 = dis * (x @ W1) ----------------
            for s in range(NSG):
                t0 = s * SG_TILES
                nt = min(NTILES, t0 + SG_TILES) - t0
                xs = xinp.tile([128, KIN, SG_TILES * 128], F16, tag="xs")
                nc.sync.dma_start(
                    xs[:, :, : nt * 128],
                    xT_v[:, :, t0 * 128 : (t0 + nt) * 128].rearrange(
                        "k p n -> p k n"
                    ),
                )
                for i in range(nt):
                    t = t0 + i
                    z_ps = mpsump.tile([128, HID], F32, tag="mm")
                    for k in range(KIN):
                        nc.tensor.matmul(
                            z_ps[:],
                            xs[:, k, i * 128 : (i + 1) * 128],
                            W1_t[:, k, :],
                            start=(k == 0),
                            stop=(k == KIN - 1),
                        )
                    nc.scalar.activation(
                        zs1_all[:, t, :],
                        z_ps[:],
                        mybir.ActivationFunctionType.Copy,
                        bias=0.0,
                        scale=dis_t[:, t : t + 1],
                    )
                nc.sync.dma_start(
                    zs1_v[t0 : t0 + nt].rearrange("g p h -> p g h"),
                    zs1_all[:, t0 : t0 + nt, :],
                )
                # quarter complete -> pipelined sub-allgather (the last
                # quarter's AG is deferred into layer 1's first supergroup so
                # its sem wait doesn't stall the first gather calls)
                for q in range(NQ):
                    if s == qsg_start[q + 1] - 1:
                        if q == NQ - 1 and os.environ.get("GCN_PAG", "1") == "1":
                            pending_ag.append(
                                lambda q=q: sub_allgather(zs1_own, tab1, q)
                            )
                        else:
                            sub_allgather(zs1_own, tab1, q)

            _xin_cm.__exit__(None, None, None)
            _gat_cm = tc.tile_pool(name="gat", bufs=5)
            gatp = _gat_cm.__enter__()
            _sel_cm = tc.tile_pool(name="sel", bufs=4)
            selp = _sel_cm.__enter__()

            # ---------------- agg layer (shared for both layers) -----------
            psums = {}

            def agg_layer(tab_dram, zs_src_all, layer):
                parts = set(
                    os.environ.get("GCN_AGG_PARTS", "gather,sel,mm,epi").split(",")
                )
                tile_seen = np.zeros(NTILES, np.int64)
                qn = 0
                for s in range(NSG):
                    t0 = s * SG_TILES
                    nt = min(NTILES, t0 + SG_TILES) - t0
                    # self-loop first (opens accumulation): aggT += zsT @ 5I
                    for i in range(nt):
                        t = t0 + i
                        ps = aggp.tile([128, 128], F32, tag="agg")
                        psums[t] = ps
                        nc.tensor.matmul(
                            ps[:],
                            zs_src_all[:, t, :],
                            diag5_t[:],
                            start=True,
                            stop=("mm" not in parts),
                        )
                    sgcalls = [c for c in calls if c[0] == s]
                    for ci, (_, b, io, co, tl) in enumerate(sgcalls):
                        if s == 0 and ci == len(sgcalls) - 1 and pending_ag:
                            pending_ag.pop()()
                        nch = sum(n for _, n in tl)
                        if nch == 0:
                            continue
                        L = nch * 128
                        idx_t = metap.tile(
                            [128, L // 16], mybir.dt.int16, tag="idx"
                        )
                        nc.sync.dma_start(
                            idx_t[:], idx_d[:, io // 16 : (io + L) // 16]
                        )
                        # Sel one-hot for the whole call in one DVE op:
                        # sel[p, j, d] = (dloc[p, co+j] == iota[d])
                        sel_t = selp.tile([128, 32, 128], F8, tag="sel")
                        if "sel" in parts:
                            in0 = bass.AP(
                                iota_t[:].tensor,
                                iota_t[:].offset,
                                [iota_t[:].ap[0], [0, nch], iota_t[:].ap[1]],
                            )
                            dl = dloc_t[:, co : co + nch]
                            in1 = bass.AP(
                                dl.tensor, dl.offset, [dl.ap[0], dl.ap[1], [0, 128]]
                            )
                            nc.vector.tensor_tensor(
                                sel_t[:, :nch, :],
                                in0,
                                in1,
                                mybir.AluOpType.is_equal,
                            )
                        else:
                            nc.vector.memset(sel_t[:, :nch, :], 0.0)
                        msg_t = gatp.tile([128, nch, HID], F16, tag="msg")
                        if "gather" in parts:
                            r0 = int(toff[b])
                            r1 = int(toff[b + 1])
                            if use_prep:
                                nc.gpsimd.dma_gather(
                                    msg_t[:],
                                    tab_dram[r0:r1, :],
                                    idx_t[:],
                                    L,
                                    L,
                                    HID,
                                    single_packet=False,
                                    queue_num=qn,
                                    prepare_only=True,
                                    sem=prep_sem(qn),
                                ).then_inc(psems[qn], 1)
                                prep_count[qn] += 1
                                nc.gpsimd.wait_ge(psems[qn], prep_count[qn])
                                nc.gpsimd.trigger_dma(count=1, queue_num=qn)
                            else:
                                nc.gpsimd.dma_gather(
                                    msg_t[:],
                                    tab_dram[r0:r1, :],
                                    idx_t[:],
                                    L,
                                    L,
                                    HID,
                                    single_packet=os.environ.get(
                                        "GCN_SP", "0"
                                    )
                                    == "1",
                                    queue_num=qn,
                                )
                            qn = (qn + 1) % NQUEUES
                        else:
                            nc.vector.memset(msg_t[:], 0.0)
                        j = 0
                        for t, n in tl:
                            for _ in range(n):
                                tile_seen[t] += 1
                                if "mm" in parts:
                                    nc.tensor.matmul(
                                        psums[t][:],
                                        msg_t[:, j, :],
                                        sel_t[:, j, :],
                                        start=False,
                                        stop=(tile_seen[t] == tile_chunks[t]),
                                    )
                                j += 1
                    # epilogue for the PREVIOUS supergroup (software
                    # pipeline: keeps the DVE/ACT epilogue ops out of the
                    # critical DVE-sel / PE-matmul overlap window)
                    if s > 0 and "epi" in parts:
                        sg_epilogue(s - 1, layer)
                if "epi" in parts:
                    sg_epilogue(NSG - 1, layer)

            def sg_epilogue(s, layer):
                t0 = s * SG_TILES
                nt = min(NTILES, t0 + SG_TILES) - t0
                o_sg = None
                if layer == 2:
                    o_sg = epip.tile([128, SG_TILES, NCLS], F32, tag="o")
                for i in range(nt):
                    epilogue(t0 + i, i, psums.pop(t0 + i), layer, o_sg)
                if layer == 1:
                    nc.sync.dma_start(
                        zs2_v[t0 : t0 + nt].rearrange("g p h -> p g h"),
                        zs2_all[:, t0 : t0 + nt, :],
                    )
                    for q in range(NQ):
                        if s == qsg_start[q + 1] - 1:
                            if q == NQ - 1 and os.environ.get("GCN_PAG", "1") == "1":
                                pending_ag.append(
                                    lambda q=q: sub_allgather(zs2_own, tab2, q)
                                )
                            else:
                                sub_allgather(zs2_own, tab2, q)
                else:
                    nc.sync.dma_start(
                        out_v[t0 : t0 + nt].rearrange("g p c -> p g c"),
                        o_sg[:, :nt, :],
                    )

            def epilogue(t, i, ps, layer, o_sg):
                if layer == 1:
                    # h1T' = relu(aggT); zs2 = dis^2 * (h1' @ W2)
                    h_sb = epip.tile([128, 128], F16, tag="h")
                    nc.vector.tensor_scalar_max(h_sb[:], ps[:], 0.0)
                    z_ps = mpsump.tile([128, HID], F32, tag="mm")
                    nc.tensor.matmul(z_ps[:], h_sb[:], W2_t[:])
                    nc.scalar.activation(
                        zs2_all[:, t, :],
                        z_ps[:],
                        mybir.ActivationFunctionType.Copy,
                        bias=0.0,
                        scale=dis2_t[:, t : t + 1],
                    )
                else:
                    # o = dis * (agg2T^T @ W3T) + b3
                    h_sb = epip.tile([128, 128], F16, tag="h")
                    nc.vector.tensor_copy(h_sb[:], ps[:])
                    o_ps = mpsump.tile([128, NCLS], F32, tag="mm")
                    nc.tensor.matmul(o_ps[:], h_sb[:], W3T_t[:])
                    nc.vector.scalar_tensor_tensor(
                        o_sg[:, i, :],
                        o_ps[:],
                        dis_t[:, t : t + 1],
                        b3r_t[:],
                        mybir.AluOpType.mult,
                        mybir.AluOpType.add,
                    )

            dbg = int(os.environ.get("GCN_DEBUG_LEVEL", "3"))
            zs2_q_dma = None  # noqa: F841

            if dbg >= 2:
                agg_layer(tab1, zs1_all, layer=1)

            if dbg >= 3:
                agg_layer(tab2, zs2_all, layer=2)
            else:
                zt = epip.tile([128, NCLS], F32, tag="zt")
                nc.vector.memset(zt[:], 0.0)
                for t in range(NTILES):
                    nc.sync.dma_start(out_d[t * 128 : (t + 1) * 128, :], zt[:])

            _sel_cm.__exit__(None, None, None)
            _gat_cm.__exit__(None, None, None)

    nc.compile()
    return nc


_PROFILE_HOOK_DONE = False


def _install_profile_hook():
    """The container's antenv lacks axon_hooks; inject it so trace=True works."""
    global _PROFILE_HOOK_DONE
    if _PROFILE_HOOK_DONE:
        return
    _PROFILE_HOOK_DONE = True
    import types

    try:
        from antenv.axon_hooks import get_axon_ntff_profile_hook  # noqa: F401

        return  # real module exists
    except ImportError:
        pass
    try:
        from trn_agent_boot.trn_boot import _ntff_profile_via_ctypes

        hook = _ntff_profile_via_ctypes("/opt/axon/libaxon_pjrt.so")
    except Exception:
        hook = None
    mod = types.ModuleType("antenv.axon_hooks")
    mod._hook = hook
    mod.set_axon_ntff_profile_hook = lambda h: setattr(mod, "_hook", h)
    mod.get_axon_ntff_profile_hook = lambda: mod._hook
    import antenv

    sys.modules["antenv.axon_hooks"] = mod
    antenv.axon_hooks = mod


def kernel(x, edge_index, W1, W2, W3, b3, trace=False):
    x = np.asarray(x)
    edge_index = np.asarray(edge_index)
    if trace:
        _install_profile_hook()
    meta, in_maps = preprocess(x, edge_index, W1, W2, W3, b3)
    nc = build_nc(meta)
    res = run_bass_kernel_spmd(nc, in_maps, list(range(NCORES)), trace=trace)
    outs = []
    for c in range(NCORES):
        o = res.results[c]["out"]  # [NPC, NCLS]
        outs.append(o[: meta["npc_raw"]])
    full = np.concatenate(outs, axis=0)[: meta["N"]]
    kernel.last_result = res
    return np.ascontiguousarray(full.astype(np.float32))


if __name__ == "__main__":
    # tiny self-test
    rng = np.random.default_rng(1)
    N, E, F, H, C = 2048, 16384, 512, 128, 16
    x = rng.standard_normal((N, F)).astype(np.float32)
    ei = rng.integers(0, N, (2, E)).astype(np.int32)
    W1 = (rng.standard_normal((F, H)) / np.sqrt(F)).astype(np.float32)
    W2 = (rng.standard_normal((H, H)) / np.sqrt(H)).astype(np.float32)
    W3 = (rng.standard_normal((C, H)) / np.sqrt(H)).astype(np.float32)
    b3 = np.zeros(C, np.float32)

    fill = float(np.trunc(np.log2(E / N)))
    deg = np.bincount(ei[0], minlength=N) + fill
    dis = 1.0 / np.sqrt(deg)

    def gcn(h, W):
        z = h @ W
        zs = dis[:, None] * z
        agg = np.zeros_like(zs)
        np.add.at(agg, ei[0], zs[ei[1]])
        return dis[:, None] * (agg + fill * zs)

    h = np.maximum(gcn(x, W1), 0.0)
    h = gcn(h, W2)
    expected = h @ W3.T + b3

    got = kernel(x, ei, W1, W2, W3, b3)
    err = np.abs(got - expected).max() / np.abs(expected).max()
    print(f"rel err: {err:.3e}")
    print("PASS" if err < 2e-2 else "FAIL")
